# revision 31
# baseline (speedup 1.0000x reference)
"""Multi-head attention kernel for Trainium2 (Bass/Tile), 8-core SPMD.

Problem: B=4, Q=K=2048, C=128, H=8, D=16 attention (dense_transformer).

Sharding: core = (batch b, head-group hg): 8 cores = 4 batches x 2 groups
of 4 heads.  Every core gets its batch's q_x/kv_x rows plus its 4 heads'
projection weights, and produces out[b, :, 4*hg:4*hg+4, :] as a contiguous
[2048, 64] block.  The host-side gather is pure numpy slicing.

Per-core algorithm (flash-attention style, transposed-scores layout):
  - PE-transpose q_x/kv_x tiles to get [c, s] layouts.
  - Project qT/kT = [d, s] per head (head h parked at partitions 32h..32h+16
    so the D=16 contraction of the score matmuls can be row-packed 4-heads
    concurrent in the 128x128 PE array), and v = [k, d] with an appended
    ones column (so the softmax denominator falls out of the AV matmul).
  - Main loop over (qblock=256) x (ktile=128): scores^T [k,q] via f32r
    matmuls, one ACT exp call per ktile over [128, 4*256] PSUM->SBUF
    (no max subtraction: scores are ~N(0,1), exp is fp32-safe), then AV
    accumulation into PSUM over all ktiles.
  - Epilogue: PE-transpose [17, 128] result blocks, reciprocal on DVE,
    scale on ACT, contiguous [128, 64] DMA stores.

Sync-slot discipline: several TRN2 instruction encodings (notably the
fp32/f32r self-loading matmul) carry only ONE embedded semaphore wait,
and Tile neither splits excess waits nor lets sequencer NOPs advance an
engine's observed-tick clock.  Also, tile-pool slot recycling attaches
multi-proc release waits to the first toucher of each recycled slot.
Therefore: (1) PSUM is managed as two persistent 4-bank tiles with
manual slice rotation (no pool recycling anywhere), and (2) tiny
single-dependency "absorber" ops on each engine (1x1 matmul on PE,
memset on DVE, 1x1 copy on ACT) observe foreign engine ticks first, so
every real matmul needs at most one embedded wait.
"""

import math
import os
import sys
from contextlib import ExitStack

import numpy as np

try:
    import concourse.bass as bass
except ImportError:  # container staging path
    sys.path.insert(0, "/opt/trn_rl_repo")
    import concourse.bass as bass

import concourse.bacc as bacc
import concourse.tile as tile
from concourse import mybir
from concourse.bass import _add_dep_helper
from concourse.bass_utils import run_bass_kernel_spmd

B, Q, KS, C, H, D = 4, 2048, 2048, 128, 8, 16
HPC = 4  # heads per core
N_CORES = 8
P = 128
NQT = Q // P  # 16
NKT = KS // P  # 16
QB = 256  # q block (columns per score matmul)
NQB = Q // QB  # 8
F32 = mybir.dt.float32
F32R = mybir.dt.float32r
K_STAGES = int(os.environ.get("K_STAGES", "3"))  # 1=setup, 2=+main, 3=+epilogue


def _dep(inst, on, reason="absorb"):
    _add_dep_helper(inst.ins, on.ins, sync=True, reason=reason)


def _after(insts, anchor, reason="phase order"):
    for i in insts:
        _add_dep_helper(i.ins, anchor.ins, sync=False, reason=reason)


def _legalize_waits(nc: bass.Bass) -> None:
    """TRN2 instruction encodings embed at most ONE semaphore wait (walrus:
    'Too many sync wait commands').  Tile can assign several.  Move excess
    waits onto a same-engine sequencer NOP inserted right before the
    instruction — the sequencer executes waits before dispatch, so the
    semantics are identical."""
    nid = [0]
    for fn in nc.m.functions:
        for blk in fn.blocks:
            out = []
            changed = False
            for inst in blk.instructions:
                si = inst.sync_info
                if (
                    si is not None
                    and si.on_wait
                    and len(si.on_wait) > 1
                    and not (
                        inst.is_sequencer_only()
                        if callable(inst.is_sequencer_only)
                        else inst.is_sequencer_only
                    )
                ):
                    for w in si.on_wait:
                        nop = mybir.InstNoOp(name=f"W-{nid[0]}", ins=[], outs=[])
                        nid[0] += 1
                        nop.engine = inst.engine
                        nop.sync_info = mybir.SyncInfo(on_wait=[w], on_update=[])
                        nc.register_instruction(nop, overwrite=True)
                        out.append(nop)
                    inst.sync_info = mybir.SyncInfo(
                        on_wait=[], on_update=list(si.on_update)
                    )
                    changed = True
                out.append(inst)
            if changed:
                blk.instructions = out


def build_attention_nc() -> bass.Bass:
    nc = bacc.Bacc()
    qx_d = nc.dram_tensor("qx", [Q, C], F32, kind="ExternalInput")
    kvx_d = nc.dram_tensor("kvx", [KS, C], F32, kind="ExternalInput")
    wq_d = nc.dram_tensor("wq", [HPC * D, C], F32, kind="ExternalInput")
    wk_d = nc.dram_tensor("wk", [HPC * D, C], F32, kind="ExternalInput")
    wv_d = nc.dram_tensor("wv", [HPC * D, C], F32, kind="ExternalInput")
    out_d = nc.dram_tensor("out", [Q, HPC * D], F32, kind="ExternalOutput")

    with tile.TileContext(nc) as tc, ExitStack() as ctx:
        const = ctx.enter_context(tc.tile_pool(name="const", bufs=1))
        sbig = ctx.enter_context(tc.tile_pool(name="sbig", bufs=1))
        psum = ctx.enter_context(tc.tile_pool(name="psum", bufs=1, space="PSUM"))

        # ---- persistent PSUM: two 4-bank tiles, manually rotated ----
        big1 = psum.tile([P, 2, HPC, QB], F32)  # scores / setup scratch / epi
        big2 = psum.tile([P, 2, HPC, QB], F32)  # AV accum / setup scratch

        identity = const.tile([P, P], F32)
        id_ms = nc.gpsimd.memset(identity, 0.0)
        id_sel = nc.gpsimd.affine_select(
            out=identity,
            in_=identity,
            compare_op=mybir.AluOpType.not_equal,
            fill=1.0,
            base=0,
            pattern=[[-1, P]],
            channel_multiplier=1,
        )
        id1 = identity[0:1, 0:1]
        zbias = const.tile([P, 1], F32)
        zb_ms = nc.vector.memset(zbias, 0.0)
        scr_src = const.tile([1, 1], F32)
        nc.vector.memset(scr_src, 0.0)
        scrd = const.tile([1, 512], F32)  # DVE absorber targets
        scra = const.tile([1, 512], F32)  # ACT absorber targets
        _ctr = [0, 0, 0]  # dve, act, pe absorber counters

        def dve_abs(on):
            i = _ctr[0]
            _ctr[0] += 1
            m = nc.vector.memset(scrd[0:1, i : i + 1], 0.0)
            _dep(m, on)
            return m

        def act_abs(on):
            i = _ctr[1]
            _ctr[1] += 1
            c = nc.scalar.copy(out=scra[0:1, i : i + 1], in_=scr_src)
            _dep(c, on)
            return c

        # PE absorbers write [1,1] into reserved columns of big2 bank 0.
        # start=False so no bank-wide pending-clear (which would create
        # bank-wide WAR deps); the columns are initialized by one start=True
        # matmul (which also makes the PE observe the identity build) so the
        # simulator never accumulates onto uninitialized PSUM.
        def pe_abs(on):
            i = _ctr[2]
            _ctr[2] += 1
            assert i < 120
            mm = nc.tensor.matmul(
                big2[0:1, 0, 0, 128 + i : 129 + i],
                lhsT=id1,
                rhs=id1,
                start=False,
                stop=False,
                skip_group_check=True,
            )
            _dep(mm, on)
            return mm

        # persistent SBUF tensors
        qxT = sbig.tile([P, NQT, P], F32R)  # [c, tile, s]
        kvxT = sbig.tile([P, NKT, P], F32R)
        qT = sbig.tile([D, HPC, Q], F32R)  # [d, h, q], q-weights pre-scaled
        kT = sbig.tile([D, HPC, KS], F32R)
        v_all = sbig.tile([P, NKT, HPC, D + 1], F32R)  # [k, ktile, h, d | one]
        o_acc = sbig.tile([D + 1, HPC, NQB, QB], F32)  # [d|sum, h, qb, q]
        et = sbig.tile([P, 3, HPC, QB], F32R)  # exp'd scores, 3-deep rotation
        r_all = sbig.tile([P, 2, HPC, 2], F32)  # 1/sum, [qb%2, h, sub]
        ofin = sbig.tile([P, 2, 2, HPC, D], F32)  # [qb%2, sub, h, d]

        wqT_sb = const.tile([P, HPC * D], F32R)  # col 16h+d = wq head h row d
        wkT_sb = const.tile([P, HPC * D], F32R)
        wvT = const.tile([P, HPC * D], F32R)

        wq_sb = sbig.tile([HPC * D, C], F32)
        wk_sb = sbig.tile([HPC * D, C], F32)
        wv_sb = sbig.tile([HPC * D, C], F32)
        qx_sb = sbig.tile([P, NQT, P], F32)
        kvx_sb = sbig.tile([P, NKT, P], F32)

        # ---------------- stage 0: load + transpose + project ----------------
        wdmas = [
            nc.sync.dma_start(out=wq_sb, in_=wq_d[:, :]),
            nc.sync.dma_start(out=wk_sb, in_=wk_d[:, :]),
            nc.sync.dma_start(out=wv_sb, in_=wv_d[:, :]),
        ]
        indmas = []
        for t in range(NQT):
            indmas.append(
                nc.sync.dma_start(out=qx_sb[:, t, :], in_=qx_d[bass.ts(t, P), :])
            )
        for t in range(NKT):
            indmas.append(
                nc.sync.dma_start(out=kvx_sb[:, t, :], in_=kvx_d[bass.ts(t, P), :])
            )
        ones_ms = nc.vector.memset(v_all[:, :, :, D : D + 1].bitcast(F32), 1.0)

        a_id = nc.tensor.matmul(
            big2[0:1, 0, 0, 128:248],
            lhsT=id1,
            rhs=identity[0:1, 0:120],
            start=True,
            stop=True,
            skip_group_check=True,
        )
        _dep(a_id, id_sel)
        wabs = [pe_abs(d) for d in wdmas]
        _after(wabs, a_id)

        # input-DMA absorbers: PE observes every input tile DMA up front
        inabs = []
        prev = wabs[-1]
        for d in indmas:
            ab = pe_abs(d)
            _after([ab], prev)
            prev = ab
            inabs.append(ab)

        # weight transposes into big2 scratch ([.., 64:128] region)
        idhd = identity[0 : HPC * D, 0 : HPC * D]
        pwq = big2[:, 0, 0, 64:128]
        pwk = big2[:, 0, 1, 64:128]
        pwv = big2[:, 0, 2, 64:128]
        tr_q = nc.tensor.transpose(pwq, wq_sb, idhd)
        tr_k = nc.tensor.transpose(pwk, wk_sb, idhd)
        tr_v = nc.tensor.transpose(pwv, wv_sb, idhd)
        _after([tr_q, tr_k, tr_v], wabs[-1])

        # one-time scatter/scale of the weight transposes on ACT
        aw2 = act_abs(ones_ms)  # ACT observes DVE memsets (first: DVE dep)
        aw1 = act_abs(tr_v)  # ACT observes PE transposes
        wmoves = [
            nc.scalar.mul(out=wqT_sb, in_=pwq, mul=1.0 / math.sqrt(D)),
            nc.scalar.copy(out=wkT_sb, in_=pwk),
            nc.scalar.copy(out=wvT, in_=pwv),
        ]
        _after(wmoves, aw1)
        _after(wmoves, aw2)

        # input transposes into big1 (8-slot rotation over [a, h] x [0:128])
        copies = []
        intrs = []
        for i in range(NQT + NKT):
            t = i if i < NQT else i - NQT
            src = qx_sb if i < NQT else kvx_sb
            dst = qxT if i < NQT else kvxT
            slot = big1[:, (i // 4) % 2, i % 4, 0:P]
            tr = nc.tensor.transpose(slot, src[:, t, :], identity)
            intrs.append(tr)
            copies.append(nc.vector.tensor_copy(dst[:, t, :], slot))
        _after(intrs, inabs[-1])

        # PE observes the ACT weight moves before projections
        b2 = pe_abs(wmoves[-1])

        setup_copies = []
        pmms = []
        # v projection: [k, d] per ktile, all heads side by side; big2
        # scratch rotation over 8 [.., 0:64] slots
        for t in range(NKT):
            pv = big2[:, t % 2, (t // 2) % 4, 0:64]
            pmms.append(
                nc.tensor.matmul(pv, lhsT=kvxT[:, t, :], rhs=wvT, start=True, stop=True)
            )
            setup_copies.append(
                nc.vector.tensor_copy(
                    v_all[:, t, :, 0:D],
                    pv.rearrange("p (h d) -> p h d", h=HPC),
                )
            )
        # projections qT / kT: per head, 4 chunks of [16, 512] into one flat
        # 4-bank scratch, then one wide ACT copy [16, 2048] per (tensor, head)
        qxT_f = qxT.rearrange("p t s -> p (t s)")
        kvxT_f = kvxT.rearrange("p t s -> p (t s)")
        b1f = big1.rearrange("p a h q -> p (a h q)")
        b2f = big2.rearrange("p a h q -> p (a h q)")
        for h in range(HPC):
            for (wT, src_f, dstT) in ((wqT_sb, qxT_f, qT), (wkT_sb, kvxT_f, kT)):
                flat = b1f if h % 2 == 0 else b2f
                for c4 in range(Q // 512):
                    pmms.append(
                        nc.tensor.matmul(
                            flat[0:D, bass.ts(c4, 512)],
                            lhsT=wT[:, bass.ts(h, D)],
                            rhs=src_f[:, bass.ts(c4, 512)],
                            start=True,
                            stop=True,
                        )
                    )
                setup_copies.append(
                    nc.scalar.copy(out=dstT[:, h, :], in_=flat[0:D, :])
                )
        _after(pmms, b2)

        # PE observes the final setup DVE tick before the main loop
        c1 = pe_abs(setup_copies[-1])

        # ---------------- stage 1: main attention loop ----------------
        o_copies = []
        qb_exps = []
        first_mms = []
        for qb in range(NQB if K_STAGES >= 2 else 0):
            qsl = bass.ts(qb, QB)
            q2 = qb % 2
            # absorber: first toucher of the recycled AV half, so the real
            # AV matmuls (1-wait-limited) only wait on the exp. Its garbage
            # write is overwritten by the start=True bank clear.
            av_pre = nc.tensor.matmul(
                big2[0:1, q2, 0, 0:1], lhsT=id1, rhs=id1,
                start=True, stop=False, skip_group_check=True,
            )
            if qb == 0:
                _after([av_pre], c1)
            av_first = {}
            av_last = {}

            def emit_st(kt):
                ps = big1[:, kt % 2, :, :]
                for h in range(HPC):
                    mm = nc.tensor.matmul(
                        ps[:, h, :],
                        lhsT=kT[:, h, bass.ts(kt, P)],
                        rhs=qT[:, h, qsl],
                        start=True,
                        stop=True,
                    )
                    if qb == 0 and kt < 2:
                        first_mms.append(mm)
                return ps

            # software pipeline: emit sT(kt+1) before AV(kt) so the PE works
            # on next scores while ACT exponentiates the current ones
            ps_cur = emit_st(0)
            for kt in range(NKT):
                ech = et[:, kt % 3, :, :]
                exp_i = nc.scalar.activation(
                    ech, ps_cur, mybir.ActivationFunctionType.Exp, bias=zbias
                )
                if kt + 1 < NKT:
                    ps_cur = emit_st(kt + 1)
                for h in range(HPC):
                    # heads {0,1} share a PSUM bank, {2,3} the next: only the
                    # even head opens the accumulation group (start clears the
                    # whole 2KB zero region), the odd head's first matmul
                    # overwrites its half via pending-zero bytes.
                    start = kt == 0 and h % 2 == 0
                    stop = kt == NKT - 1 and h % 2 == 1
                    mm = nc.tensor.matmul(
                        big2[0 : D + 1, q2, h, :],
                        lhsT=v_all[:, kt, h, :],
                        rhs=ech[:, h, :],
                        start=start,
                        stop=stop,
                    )
                    if kt == 0:
                        av_first[h] = mm
                    if kt == NKT - 1:
                        av_last[h] = mm
            qb_exps.append(exp_i)
            # enforce even-head-first ordering within each shared bank
            for h in (1, 3):
                _add_dep_helper(
                    av_first[h].ins, av_first[h - 1].ins, sync=False,
                    reason="psum zero-region open order",
                )
                _add_dep_helper(
                    av_last[h].ins, av_last[h - 1].ins, sync=False,
                    reason="psum zero-region close order",
                )
            _after(list(av_first.values()), av_pre)
            o_copies.append(
                nc.vector.tensor_copy(
                    o_acc[:, :, qb, :], big2[0 : D + 1, q2, :, :]
                )
            )
        _after(first_mms, c1)

        # ---------------- stage 2: transpose + normalize + store ----------------
        if K_STAGES < 3:
            # debug: dump qT rows so the kernel still produces output
            dbg = sbig.tile([P, 64], F32)
            dsrc = o_copies[-1] if o_copies else setup_copies[-1]
            cdbg = nc.vector.tensor_copy(dbg, qT[:, 0:64].bitcast(F32))
            _dep(cdbg, dsrc)
            for t in range(NQT):
                nc.sync.dma_start(out=out_d[bass.ts(t, P), :], in_=dbg)
            nc.compile()
            return nc
        e1 = pe_abs(o_copies[-1])
        e2 = pe_abs(qb_exps[-1])
        id17 = identity[0 : D + 1, 0 : D + 1]
        prev_dmas = {}
        prev_scales = {}
        rcs = {}
        for qb in range(NQB):
            q2 = qb % 2
            p_ab = None
            if qb >= 2:
                # dep on qb-1's reciprocal: strictly newer DVE tick than the
                # rc(qb-2) read this qb's transposes overwrite
                p_ab = pe_abs(rcs[qb - 1])
            trs = []
            for sub in range(2):
                for h in range(HPC):
                    trs.append(
                        nc.tensor.transpose(
                            big1[:, q2, h, sub * (D + 1) : (sub + 1) * (D + 1)],
                            o_acc[:, h, qb, bass.ts(sub, P)],
                            id17,
                        )
                    )
            _after(trs, e1)
            _after(trs, e2)
            if p_ab is not None:
                _after(trs, p_ab)
            # DVE observes the transposes so the reciprocal carries at most
            # one embedded wait
            f1 = dve_abs(trs[-1])
            if qb >= 2:
                dve_abs(prev_scales[qb - 2][-1])
            sums = (
                big1[:, q2, :, 0 : 2 * (D + 1)]
                .rearrange("p h (s x) -> p h s x", x=D + 1)[:, :, :, D]
            )
            rc = nc.vector.reciprocal(r_all[:, q2, :, :], sums)
            rcs[qb] = rc
            _after([rc], f1)
            # ACT observes transposes + reciprocal + recycled out-DMAs
            gouts = [act_abs(trs[-1]), act_abs(rc)]
            for dmp in prev_dmas.get(qb - 2, []):
                gouts.append(act_abs(dmp))
            scales = []
            for sub in range(2):
                for h in range(HPC):
                    scales.append(
                        nc.scalar.activation(
                            ofin[:, q2, sub, h, :],
                            big1[:, q2, h, sub * (D + 1) : sub * (D + 1) + D],
                            mybir.ActivationFunctionType.Copy,
                            scale=r_all[:, q2, h, sub : sub + 1],
                        )
                    )
            for g in gouts:
                _after(scales, g)
            prev_scales[qb] = scales
            dmas_qb = []
            for sub in range(2):
                dmas_qb.append(
                    nc.sync.dma_start(
                        out=out_d[qb * QB + sub * P : qb * QB + (sub + 1) * P, :],
                        in_=ofin[:, q2, sub, :, :],
                    )
                )
            prev_dmas[qb] = dmas_qb
    nc.compile()
    return nc


_NC = None


def _get_nc():
    global _NC
    if _NC is None:
        _NC = build_attention_nc()
    return _NC


def make_in_maps(q_x, kv_x, w_q, w_k, w_v):
    q_x = np.asarray(q_x, dtype=np.float32)
    kv_x = np.asarray(kv_x, dtype=np.float32)
    w_q = np.asarray(w_q, dtype=np.float32)
    w_k = np.asarray(w_k, dtype=np.float32)
    w_v = np.asarray(w_v, dtype=np.float32)
    in_maps = []
    for core in range(N_CORES):
        b, hg = divmod(core, 2)
        rows = slice(hg * HPC * D, (hg + 1) * HPC * D)
        in_maps.append(
            {
                "qx": np.ascontiguousarray(q_x[b]),
                "kvx": np.ascontiguousarray(kv_x[b]),
                "wq": np.ascontiguousarray(w_q[rows]),
                "wk": np.ascontiguousarray(w_k[rows]),
                "wv": np.ascontiguousarray(w_v[rows]),
            }
        )
    return in_maps


def gather_out(results):
    out = np.empty((B, Q, H, D), dtype=np.float32)
    for core in range(N_CORES):
        b, hg = divmod(core, 2)
        out[b, :, hg * HPC : (hg + 1) * HPC, :] = results[core]["out"].reshape(
            Q, HPC, D
        )
    return out


def run(q_x, kv_x, w_q, w_k, w_v, **run_kwargs):
    nc = _get_nc()
    in_maps = make_in_maps(q_x, kv_x, w_q, w_k, w_v)
    res = run_bass_kernel_spmd(nc, in_maps, list(range(N_CORES)), **run_kwargs)
    return gather_out(res.results), res


def kernel(q_x, kv_x, w_q, w_k, w_v):
    out, _ = run(q_x, kv_x, w_q, w_k, w_v)
    return out
